# revision 1
# baseline (speedup 1.0000x reference)
"""Trainium2 Bass kernel for DeepMultiBasisBilinearNet.

Strategy: pure data-parallel over the batch (16384/8 = 2048 rows per core).
All activations kept in transposed [D, B] layout on-device so that every
matmul contraction dim lands on SBUF partitions with zero on-device
transposes (host pre-transposes x and all weights). All matmuls run in
bf16 (fp32 PSUM accumulation); LayerNorm statistics are computed with
ones-vector matmuls on the tensor engine and broadcast back across
partitions with a DRAM-hop DMA.
"""

import sys

if "/opt/trn_rl_repo" not in sys.path:
    sys.path.insert(0, "/opt/trn_rl_repo")

import ml_dtypes
import numpy as np

import concourse.bass as bass
import concourse.tile as tile
from concourse import bacc, mybir
from concourse.bass_utils import run_bass_kernel_spmd

BF = mybir.dt.bfloat16
F32 = mybir.dt.float32
AF = mybir.ActivationFunctionType
ALU = mybir.AluOpType

P = 128
B, D, H, R, OUT = 16384, 1024, 4, 1024, 10
HR = H * R                 # 4096
NCORES = 8
BC = B // NCORES           # 2048 rows per core
NB = 512                   # batch tile (matmul free dim, one PSUM bank)
DC = D // P                # 8 chunks of the model dim
JC = HR // P               # 32 chunks of the bilinear dim
EGRP = 2                   # eigen-projection dout groups (PSUM pressure)
EGS = DC // EGRP           # douts per group
LN_EPS = 1e-5


def _emit_block_matmuls(nc, pools, dram, blk, acts, t):
    """Bilinear matmuls + residual for batch tile t: returns hpre chunk list.

    acts: list of 8 SBUF bf16 tiles [128, NB] holding the input in [d, b]
    layout.
    """
    sb, wp, wep_p, ip, pp, psr, pse, pst, cst = (
        pools["sb"], pools["wp"], pools["wep"], pools["ip"], pools["pp"],
        pools["ps_rl"], pools["ps_e"], pools["ps_st"], pools["const"],
    )
    br_sb = cst[f"br{blk}"]
    bl_sb = cst[f"bl{blk}"]
    be_sb = cst[f"be{blk}"]

    inter = []
    for jc in range(JC):
        wr_t = wp.tile([P, D], BF, tag="wrl")
        nc.sync.dma_start(out=wr_t[:], in_=dram[f"wr{blk}"][jc])
        wl_t = wp.tile([P, D], BF, tag="wrl")
        nc.sync.dma_start(out=wl_t[:], in_=dram[f"wl{blk}"][jc])

        ps_r = psr.tile([P, NB], F32, tag="rl")
        for dc in range(DC):
            nc.tensor.matmul(
                ps_r[:], wr_t[:, dc * P:(dc + 1) * P], acts[dc][:],
                start=(dc == 0), stop=(dc == DC - 1),
            )
        # evict right off PSUM immediately (ACT) so the bank frees fast
        tmp_r = pp.tile([P, NB], F32, tag="tmp_r")
        nc.scalar.activation(tmp_r[:], ps_r[:], AF.Identity,
                             bias=br_sb[:, jc:jc + 1])

        ps_l = psr.tile([P, NB], F32, tag="rl")
        for dc in range(DC):
            nc.tensor.matmul(
                ps_l[:], wl_t[:, dc * P:(dc + 1) * P], acts[dc][:],
                start=(dc == 0), stop=(dc == DC - 1),
            )
        tmp_l = pp.tile([P, NB], F32, tag="tmp_l")
        nc.scalar.activation(tmp_l[:], ps_l[:], AF.Identity,
                             bias=bl_sb[:, jc:jc + 1])

        it = ip.tile([P, NB], BF, tag="inter")
        nc.vector.tensor_mul(it[:], tmp_r[:], tmp_l[:])
        inter.append(it)

    # eigen projection: [D, NB] += weT[jc] @ inter[jc], in EGRP dout groups.
    # LN stats (means via 1/D-ones matmuls) are interleaved per group so the
    # post-eigen serial tail is short.
    inv_d = cst["inv_d"]       # [128, 1] bf16 filled with 1/D
    st = pst.tile([64, NB], F32, tag="st")
    hpre = []
    for g in range(EGRP):
        ps_es = [pse.tile([P, NB], F32, tag="eig", name=f"eig{i}")
                 for i in range(EGS)]
        for jc in range(JC):
            we_t = wep_p.tile([P, EGS * P], BF, tag="wep")
            nc.sync.dma_start(out=we_t[:], in_=dram[f"we{blk}"][g, jc])
            for di in range(EGS):
                nc.tensor.matmul(
                    ps_es[di][:], we_t[:, di * P:(di + 1) * P], inter[jc][:],
                    start=(jc == 0), stop=(jc == JC - 1),
                )
        for di in range(EGS):
            do = g * EGS + di
            # fused: hpre = (psum + be) + residual, bf16 out, one DVE op
            hp = sb.tile([P, NB], BF, tag="hpre", bufs=24)
            nc.vector.scalar_tensor_tensor(hp[:], ps_es[di][:],
                                           be_sb[:, do:do + 1], acts[do][:],
                                           op0=ALU.add, op1=ALU.add)
            hpre.append(hp)
        for di in range(EGS):
            do = g * EGS + di
            nc.tensor.matmul(st[0:1, :], inv_d[:, 0:1], hpre[do][:],
                             start=(do == 0), stop=(do == DC - 1))
            sq = sb.tile([P, NB], BF, tag="sq", bufs=8)
            nc.scalar.activation(sq[:], hpre[do][:], AF.Square)
            nc.tensor.matmul(st[32:33, :], inv_d[:, 0:1], sq[:],
                             start=(do == 0), stop=(do == DC - 1))
    # row-level stats inline so the stats PSUM bank frees promptly and the
    # deferred part of the layernorm needs no PSUM state:
    # a = rstd, c = -mu*rstd  (all [1, NB] f32)
    eps_t = cst["eps"]
    mu = sb.tile([1, NB], F32, tag="mu", bufs=2)
    nc.scalar.copy(mu[:], st[0:1, :])
    var = sb.tile([1, NB], F32, tag="var", bufs=2)
    nc.vector.scalar_tensor_tensor(var[:], mu[:], -1.0, mu[:],
                                   op0=ALU.mult, op1=ALU.mult)
    nc.vector.tensor_add(var[:], var[:], st[32:33, :])
    std = sb.tile([1, NB], F32, tag="std", bufs=2)
    nc.scalar.activation(std[:], var[:], AF.Sqrt, bias=eps_t[:])
    row = sb.tile([1, 2 * NB], F32, tag="row", bufs=3)
    nc.vector.reciprocal_approx_fast(out=row[:, 0:NB], in_=std[:])
    nc.vector.scalar_tensor_tensor(row[:, NB:2 * NB], mu[:], -1.0,
                                   row[:, 0:NB],
                                   op0=ALU.mult, op1=ALU.mult)  # c = -mu*rstd
    return hpre, row


def _emit_ln(nc, pools, blk, hpre_st, t):
    """Broadcast + apply of the layernorm for one block's hpre chunks."""
    hpre, row = hpre_st
    sb, pp, pse, pst, cst = (
        pools["sb"], pools["pp"], pools["ps_e"], pools["ps_st"],
        pools["const"],
    )
    g_sb = cst[f"g{blk}"]
    bb_sb = cst[f"bb{blk}"]
    ones_r = cst["ones_r"]     # [1, 128] f32 ones (broadcast lhsT)

    # broadcast [a | c] across partitions with two K=1 fp32 matmuls
    a_b = pse.tile([P, NB], F32, tag="eig", name="a_b")
    nc.tensor.matmul(a_b[:], ones_r[:, :], row[:, 0:NB], start=True, stop=True)
    c_b = pse.tile([P, NB], F32, tag="eig", name="c_b")
    nc.tensor.matmul(c_b[:], ones_r[:, :], row[:, NB:2 * NB],
                     start=True, stop=True)

    outs = []
    for do in range(DC):
        u = pp.tile([P, NB], F32, tag="u")
        nc.vector.tensor_mul(u[:], hpre[do][:], a_b[:])
        w = pp.tile([P, NB], F32, tag="w")
        nc.vector.tensor_add(w[:], u[:], c_b[:])
        ho = sb.tile([P, NB], BF, tag=f"h{blk}", bufs=16)
        nc.scalar.activation(ho[:], w[:], AF.Identity,
                             bias=bb_sb[:, do:do + 1],
                             scale=g_sb[:, do:do + 1])
        outs.append(ho)
    return outs


def build_program(bc=BC):
    """Build the per-core SPMD program. bc = rows per core."""
    nt = bc // NB
    nc = bacc.Bacc("TRN2", target_bir_lowering=False)

    dram = {
        "xT": nc.dram_tensor("xT", [D, bc], BF, kind="ExternalInput"),
        "wf": nc.dram_tensor("wf", [P, DC * OUT], BF, kind="ExternalInput"),
        "bf": nc.dram_tensor("bf", [OUT, 1], F32, kind="ExternalInput"),
        "outT": nc.dram_tensor("outT", [OUT, bc], F32, kind="ExternalOutput"),
    }
    for blk in (1, 2):
        dram[f"wr{blk}"] = nc.dram_tensor(f"wr{blk}", [JC, P, D], BF,
                                          kind="ExternalInput")
        dram[f"wl{blk}"] = nc.dram_tensor(f"wl{blk}", [JC, P, D], BF,
                                          kind="ExternalInput")
        dram[f"we{blk}"] = nc.dram_tensor(f"we{blk}", [EGRP, JC, P, EGS * P],
                                          BF, kind="ExternalInput")
        for nm, cols in ((f"br{blk}", JC), (f"bl{blk}", JC), (f"be{blk}", DC),
                         (f"g{blk}", DC), (f"bb{blk}", DC)):
            dram[nm] = nc.dram_tensor(nm, [P, cols], F32, kind="ExternalInput")

    with tile.TileContext(nc) as tc:
        with (
            tc.tile_pool(name="sb", bufs=2) as sb,
            tc.tile_pool(name="wp", bufs=6) as wp,
            tc.tile_pool(name="wep", bufs=8) as wep_p,
            tc.tile_pool(name="ip", bufs=48) as ip,
            tc.tile_pool(name="pp", bufs=3) as pp,
            tc.tile_pool(name="const", bufs=1) as cstp,
            tc.tile_pool(name="ps_rl", bufs=3, space="PSUM") as ps_rl,
            tc.tile_pool(name="ps_e", bufs=4, space="PSUM") as ps_e,
            tc.tile_pool(name="ps_st", bufs=1, space="PSUM") as ps_st,
        ):
            cst = {}
            for blk in (1, 2):
                for nm, cols in ((f"br{blk}", JC), (f"bl{blk}", JC),
                                 (f"be{blk}", DC), (f"g{blk}", DC),
                                 (f"bb{blk}", DC)):
                    cst[nm] = cstp.tile([P, cols], F32, tag=nm, name=nm)
                    nc.gpsimd.dma_start(out=cst[nm][:], in_=dram[nm][:])
            cst["inv_d"] = cstp.tile([P, 1], BF, tag="inv_d", name="inv_d")
            nc.vector.memset(cst["inv_d"][:], 1.0 / D)
            cst["ones_r"] = cstp.tile([1, P], F32, tag="ones_r", name="ones_r")
            nc.vector.memset(cst["ones_r"][:], 1.0)
            cst["eps"] = cstp.tile([1, 1], F32, tag="eps", name="eps")
            nc.vector.memset(cst["eps"][:], LN_EPS)
            cst["wf"] = cstp.tile([P, DC * OUT], BF, tag="wf", name="wf_sb")
            nc.gpsimd.dma_start(out=cst["wf"][:], in_=dram["wf"][:])
            cst["bf"] = cstp.tile([OUT, 1], F32, tag="bf", name="bf_sb")
            nc.gpsimd.dma_start(out=cst["bf"][:], in_=dram["bf"][:])

            pools = {
                "sb": sb, "wp": wp, "wep": wep_p, "ip": ip, "pp": pp,
                "const": cst, "ps_rl": ps_rl, "ps_e": ps_e, "ps_st": ps_st,
            }

            def emit_head(h2, t):
                hd = ps_e.tile([P, NB], F32, tag="eig", name="hd")
                for dc in range(DC):
                    nc.tensor.matmul(
                        hd[0:OUT, :], cst["wf"][:, dc * OUT:(dc + 1) * OUT],
                        h2[dc][:], start=(dc == 0), stop=(dc == DC - 1),
                    )
                out_sb = sb.tile([OUT, NB], F32, tag="osb")
                nc.scalar.activation(out_sb[:], hd[0:OUT, :], AF.Identity,
                                     bias=cst["bf"][:])
                nc.gpsimd.dma_start(out=dram["outT"][:, t * NB:(t + 1) * NB],
                                    in_=out_sb[:])

            # warmup: a burst of throwaway matmuls fills the initial weight-DMA
            # wait and lifts the PE HAM clock gate to 8/8 before real work
            wm_l = cstp.tile([P, P], BF, tag="wm_l", name="wm_l")
            nc.vector.memset(wm_l[:], 0.0)
            wm_r = cstp.tile([P, NB], BF, tag="wm_r", name="wm_r")
            nc.vector.memset(wm_r[:], 0.0)
            for i in range(24):
                wps = ps_rl.tile([P, NB], F32, tag="rl", name=f"warm{i}")
                nc.tensor.matmul(wps[:], wm_l[:], wm_r[:],
                                 start=True, stop=True)

            # pending = (hpre2 chunks, tile) whose block-2 layernorm + head
            # are deferred into the NEXT tile's emission so their PE ops fill
            # bubbles under that tile's matmul stream instead of making one.
            pending = None
            for t in range(nt):
                x_bf = []
                for dc in range(DC):
                    xt = sb.tile([P, NB], BF, tag="xbf", bufs=16)
                    nc.sync.dma_start(
                        out=xt[:],
                        in_=dram["xT"][dc * P:(dc + 1) * P,
                                       t * NB:(t + 1) * NB],
                    )
                    x_bf.append(xt)

                hpre1 = _emit_block_matmuls(nc, pools, dram, 1, x_bf, t)
                h1 = _emit_ln(nc, pools, 1, hpre1, t)
                if pending is not None:
                    hpre2_prev, t_prev = pending
                    h2_prev = _emit_ln(nc, pools, 2, hpre2_prev, t_prev)
                    emit_head(h2_prev, t_prev)
                hpre2 = _emit_block_matmuls(nc, pools, dram, 2, h1, t)
                pending = (hpre2, t)
            hpre2_prev, t_prev = pending
            emit_head(_emit_ln(nc, pools, 2, hpre2_prev, t_prev), t_prev)
    nc.compile()
    return nc


def _bf(a):
    return np.ascontiguousarray(a.astype(ml_dtypes.bfloat16))


def prep_inputs(inputs, bc=BC, ncores=NCORES):
    """Host-side shard + transpose + bf16 conversion. Returns in_maps."""
    f = {k: np.asarray(v, dtype=np.float32) for k, v in inputs.items()}

    shared = {}
    for blk in (1, 2):
        for side in ("r", "l"):
            w = f[f"w{side}{blk}"].reshape(HR, D)          # [j, d]
            panel = w.reshape(JC, P, DC, P).transpose(0, 3, 2, 1)
            shared[f"w{side}{blk}"] = _bf(panel.reshape(JC, P, D))
            shared[f"b{side}{blk}"] = np.ascontiguousarray(
                f[f"b{side}{blk}"].reshape(JC, P).T)        # [128, 32]
        weT = f[f"we{blk}"].T                               # [j, d_out]
        panel = weT.reshape(JC, P, EGRP, EGS * P).transpose(2, 0, 1, 3)
        shared[f"we{blk}"] = _bf(panel)                     # [g, jc, p, 512]
        shared[f"be{blk}"] = np.ascontiguousarray(
            f[f"be{blk}"].reshape(DC, P).T)                 # [128, 8]
        shared[f"g{blk}"] = np.ascontiguousarray(
            f[f"g{blk}"].reshape(DC, P).T)
        shared[f"bb{blk}"] = np.ascontiguousarray(
            f[f"b{blk}"].reshape(DC, P).T)
    shared["wf"] = _bf(f["wf"].T.reshape(DC, P, OUT).transpose(1, 0, 2)
                       .reshape(P, DC * OUT))               # [128, 80]
    shared["bf"] = np.ascontiguousarray(f["bf"].reshape(OUT, 1))

    x = f["x"]
    in_maps = []
    for c in range(ncores):
        m = dict(shared)
        m["xT"] = _bf(x[c * bc:(c + 1) * bc].T)             # [1024, bc]
        in_maps.append(m)
    return in_maps


_PROGRAM_CACHE = {}


def get_program(bc=BC):
    if bc not in _PROGRAM_CACHE:
        _PROGRAM_CACHE[bc] = build_program(bc)
    return _PROGRAM_CACHE[bc]


def kernel(**inputs):
    nc = get_program(BC)
    in_maps = prep_inputs(inputs, BC, NCORES)
    res = run_bass_kernel_spmd(nc, in_maps, core_ids=list(range(NCORES)))
    out = np.concatenate([res.results[c]["outT"] for c in range(NCORES)],
                         axis=1).T
    return np.ascontiguousarray(out.astype(np.float32))


if __name__ == "__main__":
    raise SystemExit("import kernel and call kernel(**inputs); see test.py")



# revision 2
# speedup vs baseline: 1.1808x; 1.1808x over previous
"""Trainium2 Bass kernel for DeepMultiBasisBilinearNet.

Strategy: pure data-parallel over the batch (16384/8 = 2048 rows per core).
All activations kept in transposed [D, B] layout on-device so that every
matmul contraction dim lands on SBUF partitions with zero on-device
transposes (host pre-transposes x and all weights). All matmuls run in
bf16 (fp32 PSUM accumulation); LayerNorm statistics are computed with
ones-vector matmuls on the tensor engine and broadcast back across
partitions with a DRAM-hop DMA.
"""

import sys

if "/opt/trn_rl_repo" not in sys.path:
    sys.path.insert(0, "/opt/trn_rl_repo")

import ml_dtypes
import numpy as np

import concourse.bass as bass
import concourse.tile as tile
from concourse import bacc, mybir
from concourse.bass_utils import run_bass_kernel_spmd

BF = mybir.dt.bfloat16
F32 = mybir.dt.float32
AF = mybir.ActivationFunctionType
ALU = mybir.AluOpType

P = 128
B, D, H, R, OUT = 16384, 1024, 4, 1024, 10
HR = H * R                 # 4096
NCORES = 8
BC = B // NCORES           # 2048 rows per core
NB = 512                   # batch tile (matmul free dim, one PSUM bank)
DC = D // P                # 8 chunks of the model dim
JC = HR // P               # 32 chunks of the bilinear dim
EGRP = 2                   # eigen-projection dout groups (PSUM pressure)
EGS = DC // EGRP           # douts per group
LN_EPS = 1e-5


def _emit_block_matmuls(nc, pools, dram, blk, acts, t):
    """Bilinear matmuls + residual for batch tile t: returns hpre chunk list.

    acts: list of 8 SBUF bf16 tiles [128, NB] holding the input in [d, b]
    layout.
    """
    sb, wp, wep_p, ip, pp, psr, pse, pst, cst = (
        pools["sb"], pools["wp"], pools["wep"], pools["ip"], pools["pp"],
        pools["ps_rl"], pools["ps_e"], pools["ps_st"], pools["const"],
    )
    br_sb = cst[f"br{blk}"]
    bl_sb = cst[f"bl{blk}"]
    be_sb = cst[f"be{blk}"]

    inter = []
    for jc in range(JC):
        wr_t = wp.tile([P, D], BF, tag="wrl")
        nc.sync.dma_start(out=wr_t[:], in_=dram[f"wr{blk}"][jc])
        wl_t = wp.tile([P, D], BF, tag="wrl")
        nc.sync.dma_start(out=wl_t[:], in_=dram[f"wl{blk}"][jc])

        # r and l chains INTERLEAVED: consecutive matmuls alternate PSUM
        # banks, hiding the PE drain + weight-load serialization that a
        # same-bank accumulation chain incurs (~200 cycles/matmul).
        ps_r = psr.tile([P, NB], F32, tag="rl")
        ps_l = psr.tile([P, NB], F32, tag="rl")
        for dc in range(DC):
            nc.tensor.matmul(
                ps_r[:], wr_t[:, dc * P:(dc + 1) * P], acts[dc][:],
                start=(dc == 0), stop=(dc == DC - 1),
            )
            nc.tensor.matmul(
                ps_l[:], wl_t[:, dc * P:(dc + 1) * P], acts[dc][:],
                start=(dc == 0), stop=(dc == DC - 1),
            )
        # evict right off PSUM immediately (ACT) so the bank frees fast
        tmp_r = pp.tile([P, NB], F32, tag="tmp_r")
        nc.scalar.activation(tmp_r[:], ps_r[:], AF.Identity,
                             bias=br_sb[:, jc:jc + 1])
        tmp_l = pp.tile([P, NB], F32, tag="tmp_l")
        nc.scalar.activation(tmp_l[:], ps_l[:], AF.Identity,
                             bias=bl_sb[:, jc:jc + 1])

        it = ip.tile([P, NB], BF, tag="inter")
        nc.vector.tensor_mul(it[:], tmp_r[:], tmp_l[:])
        inter.append(it)

    # eigen projection: [D, NB] += weT[jc] @ inter[jc], in EGRP dout groups.
    # LN stats (means via 1/D-ones matmuls) are interleaved per group so the
    # post-eigen serial tail is short.
    inv_d = cst["inv_d"]       # [128, 1] bf16 filled with 1/D
    st = pst.tile([64, NB], F32, tag="st")
    hpre = []
    for g in range(EGRP):
        ps_es = [pse.tile([P, NB], F32, tag="eig", name=f"eig{i}")
                 for i in range(EGS)]
        for jc in range(JC):
            we_t = wep_p.tile([P, EGS * P], BF, tag="wep")
            nc.sync.dma_start(out=we_t[:], in_=dram[f"we{blk}"][g, jc])
            for di in range(EGS):
                nc.tensor.matmul(
                    ps_es[di][:], we_t[:, di * P:(di + 1) * P], inter[jc][:],
                    start=(jc == 0), stop=(jc == JC - 1),
                )
        for di in range(EGS):
            do = g * EGS + di
            # fused: hpre = (psum + be) + residual, bf16 out, one DVE op
            hp = sb.tile([P, NB], BF, tag="hpre", bufs=24)
            nc.vector.scalar_tensor_tensor(hp[:], ps_es[di][:],
                                           be_sb[:, do:do + 1], acts[do][:],
                                           op0=ALU.add, op1=ALU.add)
            hpre.append(hp)
        for di in range(EGS):
            do = g * EGS + di
            nc.tensor.matmul(st[0:1, :], inv_d[:, 0:1], hpre[do][:],
                             start=(do == 0), stop=(do == DC - 1))
            sq = sb.tile([P, NB], BF, tag="sq", bufs=8)
            nc.scalar.activation(sq[:], hpre[do][:], AF.Square)
            nc.tensor.matmul(st[32:33, :], inv_d[:, 0:1], sq[:],
                             start=(do == 0), stop=(do == DC - 1))
    # row-level stats inline so the stats PSUM bank frees promptly and the
    # deferred part of the layernorm needs no PSUM state:
    # a = rstd, c = -mu*rstd  (all [1, NB] f32)
    eps_t = cst["eps"]
    mu = sb.tile([1, NB], F32, tag="mu", bufs=2)
    nc.scalar.copy(mu[:], st[0:1, :])
    var = sb.tile([1, NB], F32, tag="var", bufs=2)
    nc.vector.scalar_tensor_tensor(var[:], mu[:], -1.0, mu[:],
                                   op0=ALU.mult, op1=ALU.mult)
    nc.vector.tensor_add(var[:], var[:], st[32:33, :])
    std = sb.tile([1, NB], F32, tag="std", bufs=2)
    nc.scalar.activation(std[:], var[:], AF.Sqrt, bias=eps_t[:])
    row = sb.tile([1, 2 * NB], F32, tag="row", bufs=3)
    nc.vector.reciprocal_approx_fast(out=row[:, 0:NB], in_=std[:])
    nc.vector.scalar_tensor_tensor(row[:, NB:2 * NB], mu[:], -1.0,
                                   row[:, 0:NB],
                                   op0=ALU.mult, op1=ALU.mult)  # c = -mu*rstd
    return hpre, row


def _emit_ln(nc, pools, blk, hpre_st, t):
    """Broadcast + apply of the layernorm for one block's hpre chunks."""
    hpre, row = hpre_st
    sb, pp, pse, pst, cst = (
        pools["sb"], pools["pp"], pools["ps_e"], pools["ps_st"],
        pools["const"],
    )
    g_sb = cst[f"g{blk}"]
    bb_sb = cst[f"bb{blk}"]
    ones_r = cst["ones_r"]     # [1, 128] f32 ones (broadcast lhsT)

    # broadcast [a | c] across partitions with two K=1 fp32 matmuls
    a_b = pse.tile([P, NB], F32, tag="eig", name="a_b")
    nc.tensor.matmul(a_b[:], ones_r[:, :], row[:, 0:NB], start=True, stop=True)
    c_b = pse.tile([P, NB], F32, tag="eig", name="c_b")
    nc.tensor.matmul(c_b[:], ones_r[:, :], row[:, NB:2 * NB],
                     start=True, stop=True)

    outs = []
    for do in range(DC):
        u = pp.tile([P, NB], F32, tag="u")
        nc.vector.tensor_mul(u[:], hpre[do][:], a_b[:])
        w = pp.tile([P, NB], F32, tag="w")
        nc.vector.tensor_add(w[:], u[:], c_b[:])
        ho = sb.tile([P, NB], BF, tag=f"h{blk}", bufs=16)
        nc.scalar.activation(ho[:], w[:], AF.Identity,
                             bias=bb_sb[:, do:do + 1],
                             scale=g_sb[:, do:do + 1])
        outs.append(ho)
    return outs


def build_program(bc=BC):
    """Build the per-core SPMD program. bc = rows per core."""
    nt = bc // NB
    nc = bacc.Bacc("TRN2", target_bir_lowering=False)

    dram = {
        "xT": nc.dram_tensor("xT", [D, bc], BF, kind="ExternalInput"),
        "wf": nc.dram_tensor("wf", [P, DC * OUT], BF, kind="ExternalInput"),
        "bf": nc.dram_tensor("bf", [OUT, 1], F32, kind="ExternalInput"),
        "outT": nc.dram_tensor("outT", [OUT, bc], F32, kind="ExternalOutput"),
    }
    for blk in (1, 2):
        dram[f"wr{blk}"] = nc.dram_tensor(f"wr{blk}", [JC, P, D], BF,
                                          kind="ExternalInput")
        dram[f"wl{blk}"] = nc.dram_tensor(f"wl{blk}", [JC, P, D], BF,
                                          kind="ExternalInput")
        dram[f"we{blk}"] = nc.dram_tensor(f"we{blk}", [EGRP, JC, P, EGS * P],
                                          BF, kind="ExternalInput")
        for nm, cols in ((f"br{blk}", JC), (f"bl{blk}", JC), (f"be{blk}", DC),
                         (f"g{blk}", DC), (f"bb{blk}", DC)):
            dram[nm] = nc.dram_tensor(nm, [P, cols], F32, kind="ExternalInput")

    with tile.TileContext(nc) as tc:
        with (
            tc.tile_pool(name="sb", bufs=2) as sb,
            tc.tile_pool(name="wp", bufs=6) as wp,
            tc.tile_pool(name="wep", bufs=8) as wep_p,
            tc.tile_pool(name="ip", bufs=48) as ip,
            tc.tile_pool(name="pp", bufs=3) as pp,
            tc.tile_pool(name="const", bufs=1) as cstp,
            tc.tile_pool(name="ps_rl", bufs=3, space="PSUM") as ps_rl,
            tc.tile_pool(name="ps_e", bufs=4, space="PSUM") as ps_e,
            tc.tile_pool(name="ps_st", bufs=1, space="PSUM") as ps_st,
        ):
            cst = {}
            for blk in (1, 2):
                for nm, cols in ((f"br{blk}", JC), (f"bl{blk}", JC),
                                 (f"be{blk}", DC), (f"g{blk}", DC),
                                 (f"bb{blk}", DC)):
                    cst[nm] = cstp.tile([P, cols], F32, tag=nm, name=nm)
                    nc.gpsimd.dma_start(out=cst[nm][:], in_=dram[nm][:])
            cst["inv_d"] = cstp.tile([P, 1], BF, tag="inv_d", name="inv_d")
            nc.vector.memset(cst["inv_d"][:], 1.0 / D)
            cst["ones_r"] = cstp.tile([1, P], F32, tag="ones_r", name="ones_r")
            nc.vector.memset(cst["ones_r"][:], 1.0)
            cst["eps"] = cstp.tile([1, 1], F32, tag="eps", name="eps")
            nc.vector.memset(cst["eps"][:], LN_EPS)
            cst["wf"] = cstp.tile([P, DC * OUT], BF, tag="wf", name="wf_sb")
            nc.gpsimd.dma_start(out=cst["wf"][:], in_=dram["wf"][:])
            cst["bf"] = cstp.tile([OUT, 1], F32, tag="bf", name="bf_sb")
            nc.gpsimd.dma_start(out=cst["bf"][:], in_=dram["bf"][:])

            pools = {
                "sb": sb, "wp": wp, "wep": wep_p, "ip": ip, "pp": pp,
                "const": cst, "ps_rl": ps_rl, "ps_e": ps_e, "ps_st": ps_st,
            }

            def emit_head(h2, t):
                hd = ps_e.tile([P, NB], F32, tag="eig", name="hd")
                for dc in range(DC):
                    nc.tensor.matmul(
                        hd[0:OUT, :], cst["wf"][:, dc * OUT:(dc + 1) * OUT],
                        h2[dc][:], start=(dc == 0), stop=(dc == DC - 1),
                    )
                out_sb = sb.tile([OUT, NB], F32, tag="osb")
                nc.scalar.activation(out_sb[:], hd[0:OUT, :], AF.Identity,
                                     bias=cst["bf"][:])
                nc.gpsimd.dma_start(out=dram["outT"][:, t * NB:(t + 1) * NB],
                                    in_=out_sb[:])

            # warmup: a burst of throwaway matmuls fills the initial weight-DMA
            # wait and lifts the PE HAM clock gate to 8/8 before real work
            wm_l = cstp.tile([P, P], BF, tag="wm_l", name="wm_l")
            nc.vector.memset(wm_l[:], 0.0)
            wm_r = cstp.tile([P, NB], BF, tag="wm_r", name="wm_r")
            nc.vector.memset(wm_r[:], 0.0)
            for i in range(24):
                wps = ps_rl.tile([P, NB], F32, tag="rl", name=f"warm{i}")
                nc.tensor.matmul(wps[:], wm_l[:], wm_r[:],
                                 start=True, stop=True)

            # pending = (hpre2 chunks, tile) whose block-2 layernorm + head
            # are deferred into the NEXT tile's emission so their PE ops fill
            # bubbles under that tile's matmul stream instead of making one.
            pending = None
            for t in range(nt):
                x_bf = []
                for dc in range(DC):
                    xt = sb.tile([P, NB], BF, tag="xbf", bufs=16)
                    nc.sync.dma_start(
                        out=xt[:],
                        in_=dram["xT"][dc * P:(dc + 1) * P,
                                       t * NB:(t + 1) * NB],
                    )
                    x_bf.append(xt)

                hpre1 = _emit_block_matmuls(nc, pools, dram, 1, x_bf, t)
                h1 = _emit_ln(nc, pools, 1, hpre1, t)
                if pending is not None:
                    hpre2_prev, t_prev = pending
                    h2_prev = _emit_ln(nc, pools, 2, hpre2_prev, t_prev)
                    emit_head(h2_prev, t_prev)
                hpre2 = _emit_block_matmuls(nc, pools, dram, 2, h1, t)
                pending = (hpre2, t)
            hpre2_prev, t_prev = pending
            emit_head(_emit_ln(nc, pools, 2, hpre2_prev, t_prev), t_prev)
    nc.compile()
    return nc


def _bf(a):
    return np.ascontiguousarray(a.astype(ml_dtypes.bfloat16))


def prep_inputs(inputs, bc=BC, ncores=NCORES):
    """Host-side shard + transpose + bf16 conversion. Returns in_maps."""
    f = {k: np.asarray(v, dtype=np.float32) for k, v in inputs.items()}

    shared = {}
    for blk in (1, 2):
        for side in ("r", "l"):
            w = f[f"w{side}{blk}"].reshape(HR, D)          # [j, d]
            panel = w.reshape(JC, P, DC, P).transpose(0, 3, 2, 1)
            shared[f"w{side}{blk}"] = _bf(panel.reshape(JC, P, D))
            shared[f"b{side}{blk}"] = np.ascontiguousarray(
                f[f"b{side}{blk}"].reshape(JC, P).T)        # [128, 32]
        weT = f[f"we{blk}"].T                               # [j, d_out]
        panel = weT.reshape(JC, P, EGRP, EGS * P).transpose(2, 0, 1, 3)
        shared[f"we{blk}"] = _bf(panel)                     # [g, jc, p, 512]
        shared[f"be{blk}"] = np.ascontiguousarray(
            f[f"be{blk}"].reshape(DC, P).T)                 # [128, 8]
        shared[f"g{blk}"] = np.ascontiguousarray(
            f[f"g{blk}"].reshape(DC, P).T)
        shared[f"bb{blk}"] = np.ascontiguousarray(
            f[f"b{blk}"].reshape(DC, P).T)
    shared["wf"] = _bf(f["wf"].T.reshape(DC, P, OUT).transpose(1, 0, 2)
                       .reshape(P, DC * OUT))               # [128, 80]
    shared["bf"] = np.ascontiguousarray(f["bf"].reshape(OUT, 1))

    x = f["x"]
    in_maps = []
    for c in range(ncores):
        m = dict(shared)
        m["xT"] = _bf(x[c * bc:(c + 1) * bc].T)             # [1024, bc]
        in_maps.append(m)
    return in_maps


_PROGRAM_CACHE = {}


def get_program(bc=BC):
    if bc not in _PROGRAM_CACHE:
        _PROGRAM_CACHE[bc] = build_program(bc)
    return _PROGRAM_CACHE[bc]


def kernel(**inputs):
    nc = get_program(BC)
    in_maps = prep_inputs(inputs, BC, NCORES)
    res = run_bass_kernel_spmd(nc, in_maps, core_ids=list(range(NCORES)))
    out = np.concatenate([res.results[c]["outT"] for c in range(NCORES)],
                         axis=1).T
    return np.ascontiguousarray(out.astype(np.float32))


if __name__ == "__main__":
    raise SystemExit("import kernel and call kernel(**inputs); see test.py")

